# revision 29
# baseline (speedup 1.0000x reference)
"""NF4-packed embedding lookup kernel for 8 Trainium2 NeuronCores.

Strategy (vocab-parallel, nibble-keyed act-table decode):
  - The packed table rows are sharded across the 8 cores (6283 rows each);
    tokens are routed on host to the owning core and deduplicated, so each
    core dequantizes only its unique rows. Total per-core DMA is the
    bottleneck (~330 GB/s shared between reads and writes), so the table
    stays in its dense int8 form (2KB/row reads) and the output is fp16
    (8KB/row writes).
  - On device: dma_gather int8 rows; DVE extracts the four nibble planes
    of each int16 word pair with 2-op tensor_scalars ((w & mask) << s ->
    nibble<<7, i.e. bf16 with the nibble in the exponent field), running
    in the 4x 16-bit perf mode.
  - One fully-contiguous activation instruction per tile dequantizes all
    four planes: the sqrt act table is patched so that input
    2^(nib-1) + 0.5 (scale=2^126, bias=0.5 -> exponent nib-1, never zero)
    returns lut[nib]/c on every mantissa bucket, emitted as fp16. Rows
    leave the device plane-blocked ([4, 1024] per row); output DMAs
    alternate between the SP and ACT DGE queues.
  - Host de-interleaves columns (col 4m+f <- [f, m]), casts fp16 -> f32,
    and scatters rows back to original token order.
"""

import json
import math
import os
import shutil
import sys
import tempfile

sys.path.insert(0, "/opt/trn_rl_repo")

import numpy as np

import concourse.bass as bass
import concourse.tile as tile
from concourse import bacc, mybir
from concourse import bass_utils

N_CORES = 8
P = 128          # SBUF partitions
GROUP = 256      # rows per dma_gather / activation tile (2 slots of 128)


def _sqrt_profiles(src_dir):
    """Yield (ent, prof, fe, width) for every act set containing sqrt."""
    info = json.load(open(os.path.join(src_dir, "act_info.json")))
    for ent in info["act_func_sets"]:
        if "sqrt" not in ent["act"]:
            continue
        prof = json.load(open(os.path.join(src_dir, ent["profile_json"])))
        fe = prof.get("func_exp_to_bkt_start_idx", {}).get("sqrt")
        if not fe:
            continue
        ks = sorted(int(k) for k in fe.keys())
        width = {}
        for i in range(len(ks) - 1):
            width[ks[i]] = fe[str(ks[i + 1])][0] - fe[str(ks[i])][0]
        yield ent, prof, fe, width


def _make_patched_act_dir(dst_dir, values16):
    """Copy gen3 act tables; patch every sqrt set so that an input with
    exponent e in [-1, 14] returns values16[e+1] on every mantissa bucket
    (device inputs are 2^(nib-1) + 0.5). fzero/neg-region slots are also
    set defensively."""
    from concourse.nix import assert_in_nix_environment

    assert_in_nix_environment()
    from neuronxcc.driver.Job import Job
    from neuronxcc.driver.jobs.support.FindActInfo import findActInfoFile

    src_dir = os.path.dirname(findActInfoFile(Job.getPackageDir(), "gen3"))
    os.makedirs(dst_dir, exist_ok=True)
    for fn in os.listdir(src_dir):
        shutil.copy(os.path.join(src_dir, fn), os.path.join(dst_dir, fn))
        os.chmod(os.path.join(dst_dir, fn), 0o644)

    patched = []
    for ent, prof, fe, width in _sqrt_profiles(dst_dir):
        bkt_path = os.path.join(dst_dir, ent["bkt_bin"])
        a = (
            np.frombuffer(open(bkt_path, "rb").read(), dtype=np.float32)
            .reshape(-1, 8)
            .copy()
        )
        for e in range(-1, 15):
            b0 = fe[str(e)][0]
            nb = width[e]
            a[b0 : b0 + nb, 0] = np.float32(values16[e + 1])
            a[b0 : b0 + nb, 1:4] = 0.0
        open(bkt_path, "wb").write(a.astype(np.float32).tobytes())
        for m in prof["profile_meta_data"]:
            if m["func_name"].startswith("sqrt"):
                m["fzero_result"] = int(np.float32(values16[0]).view(np.uint32))
        json.dump(prof, open(os.path.join(dst_dir, ent["profile_json"]), "w"))
        patched.append(ent["name"])
    assert patched, "no sqrt act tables found to patch"
    return os.path.join(dst_dir, "act_info.json")


def _build_program(shard_rows, cap, tag, reps=1):
    """Per-core Bass program: gather GROUP int8 rows, extract nibble planes
    on DVE, one activation (patched sqrt, scale 2^126 bias 0.5) per group
    -> fp16 rows in natural column order, DMA out."""
    d_half = 2048  # packed bytes per row
    d = 2 * d_half
    w = d_half // 2  # int16 words per row
    n_groups = cap // GROUP
    assert cap % GROUP == 0

    nc = bacc.Bacc(
        "TRN2",
        target_bir_lowering=False,
        debug=False,
        enable_asserts=False,
        num_devices=N_CORES,
        num_swdge_queues=2,
    )
    table = nc.dram_tensor(
        "table", [shard_rows, d_half], mybir.dt.int8, kind="ExternalInput"
    ).ap()
    idxs_name = f"idxs_{tag}"
    idxs = nc.dram_tensor(
        idxs_name, [P, cap // 16], mybir.dt.int16, kind="ExternalInput"
    ).ap()
    out = nc.dram_tensor(
        "out", [cap, d], mybir.dt.float16, kind="ExternalOutput"
    ).ap()

    f16 = mybir.dt.float16
    bf16 = mybir.dt.bfloat16
    i16 = mybir.dt.int16
    i8 = mybir.dt.int8
    Alu = mybir.AluOpType
    S = GROUP // P  # row slots per group

    # word w = (byte 2m, byte 2m+1); output col 4m+f:
    #   f=0: hi(b_2m)   = (w & 0x00F0) << 3
    #   f=1: lo(b_2m)   = (w & 0x000F) << 7
    #   f=2: hi(b_2m+1) = (w & 0xF000) >> 5 (logical)
    #   f=3: lo(b_2m+1) = (w & 0x0F00) >> 1 (logical)
    PLANES = [
        (0x00F0, 3, Alu.logical_shift_left),
        (0x000F, 7, Alu.logical_shift_left),
        (-4096, 5, Alu.logical_shift_right),   # 0xF000 as int16
        (0x0F00, 1, Alu.logical_shift_right),
    ]

    with tile.TileContext(nc) as tc:
        with (
            tc.tile_pool(name="idxp", bufs=1) as idxp,
            tc.tile_pool(name="gp", bufs=4) as gp,
            tc.tile_pool(name="pp", bufs=4) as pp,
            tc.tile_pool(name="op", bufs=4) as outp,
        ):
            idxt = idxp.tile([P, cap // 16], i16)
            nc.sync.dma_start(idxt[:], idxs[:])
            biast = idxp.tile([P, 1], mybir.dt.float32)
            nc.vector.memset(biast[:], 0.5)

            for g in [gg % n_groups for gg in range(reps * n_groups)]:
                gt = gp.tile([P, S, d_half], i8, tag="g")
                nc.gpsimd.dma_gather(
                    gt[:],
                    table[:],
                    idxt[:, g * (GROUP // 16) : (g + 1) * (GROUP // 16)],
                    num_idxs=GROUP,
                    num_idxs_reg=GROUP,
                    elem_size=d_half,
                    elem_step=d_half,
                    queue_num=g % 2,
                )
                wt = gt[:].bitcast(i16)  # [P, S, w]
                pt = pp.tile([P, S, 4, w], i16, tag="pt")
                for f, (mask, sh, shop) in enumerate(PLANES):
                    nc.vector.tensor_scalar(
                        pt[:, :, f, :], wt, mask, sh, Alu.bitwise_and, shop
                    )
                # single fully-contiguous activation; rows come out
                # plane-blocked ([4, 1024] per slot) and the host
                # de-interleaves columns during assembly.
                ot = outp.tile([P, S, 4, w], f16, tag="ot")
                nc.scalar.activation(
                    ot[:],
                    pt[:].bitcast(bf16),
                    mybir.ActivationFunctionType.Sqrt,
                    scale=float(2.0 ** 126),
                    bias=biast[:],
                )
                # out rows g*GROUP + s*128 + p  <-  ot[p, s, :]
                dst = out[g * GROUP : (g + 1) * GROUP, :].rearrange(
                    "(s p) e -> p s e", p=P
                )
                # alternate the issuing engine so output DMAs ride two
                # independent DGE queues
                eng = nc.sync if g % 2 == 0 else nc.scalar
                eng.dma_start(dst, ot[:])

    nc.compile()
    return nc


def _prepare(x, packed, nf4_lut, c, reps=1):
    """Host-side sharding + table encoding. Returns (nc, in_maps, meta)."""
    x = np.asarray(x)
    packed = np.asarray(packed)
    nf4_lut = np.asarray(nf4_lut, dtype=np.float32)
    c = np.asarray(c, dtype=np.float32)

    v, d_half = packed.shape
    d = 2 * d_half
    flat = x.ravel().astype(np.int64)
    n_tok = flat.size

    shard_rows = math.ceil(v / N_CORES)
    core_of = flat // shard_rows
    rel = (flat % shard_rows).astype(np.int16)

    order = np.argsort(core_of, kind="stable")
    counts = np.bincount(core_of, minlength=N_CORES)

    # exact f32 semantics of reference: nf4_lut[idx] / c
    scaled = (nf4_lut / np.float32(c[0])).astype(np.float32)
    values16 = [float(scaled[k]) for k in range(16)]

    act_dir = tempfile.mkdtemp(prefix="act_nib_")
    os.environ["BASS_ACT_ROOT_JSON_PATH"] = _make_patched_act_dir(act_dir, values16)

    import hashlib

    tag = hashlib.sha1(
        np.asarray(values16, np.float32).tobytes() + f"r{reps}v5".encode()
    ).hexdigest()[:12]
    idxs_name = f"idxs_{tag}"

    # dense int8 table, pad rows to uniform shard size
    enc = (np.asarray(packed).astype(np.int64) & 0xFF).astype(np.uint8)
    pad_rows = shard_rows * N_CORES - v
    if pad_rows:
        enc = np.concatenate([enc, np.zeros((pad_rows, d_half), np.uint8)], axis=0)
    enc = enc.view(np.int8)

    in_maps = []
    per_core_positions = []
    per_core_inv = []
    uniq_lists = []
    start = 0
    for ci in range(N_CORES):
        cnt = int(counts[ci])
        pos = order[start : start + cnt]
        start += cnt
        per_core_positions.append(pos)
        uniq, inv = np.unique(rel[pos], return_inverse=True)
        uniq_lists.append(uniq.astype(np.int16))
        per_core_inv.append(inv)
    n_uniq = [len(u) for u in uniq_lists]
    cap = max(GROUP, math.ceil(max(n_uniq) / GROUP) * GROUP)
    for ci in range(N_CORES):
        uniq = uniq_lists[ci]
        rel_ids = np.zeros(cap, dtype=np.int16)
        rel_ids[: len(uniq)] = uniq
        wrapped = rel_ids.reshape(cap // 16, 16).T  # [16, cap//16]
        idx_arr = np.tile(wrapped, (8, 1))  # replicate to 128 partitions
        in_maps.append(
            {
                "table": np.ascontiguousarray(
                    enc[ci * shard_rows : (ci + 1) * shard_rows]
                ),
                idxs_name: np.ascontiguousarray(idx_arr),
            }
        )

    nc = _build_program(shard_rows, cap, tag, reps=reps)

    meta = {
        "counts": counts,
        "positions": per_core_positions,
        "inv": per_core_inv,
        "n_tok": n_tok,
        "d": d,
        "x_shape": x.shape,
    }
    return nc, in_maps, meta


def _assemble(meta, per_core_raw):
    """per_core_raw: list of [cap, d] fp16 per core (rows plane-blocked:
    row = [f, m] with true col 4m+f) -> full [x_shape..., d] f32 output."""
    d = meta["d"]
    out_flat = np.empty((meta["n_tok"], d), dtype=np.float32)
    for ci in range(len(per_core_raw)):
        inv = meta["inv"][ci]
        rows16 = np.asarray(per_core_raw[ci]).view(np.float16).reshape(-1, d)
        sel = rows16[inv].reshape(-1, 4, d // 4).transpose(0, 2, 1).reshape(-1, d)
        out_flat[meta["positions"][ci]] = sel.astype(np.float32)
    return out_flat.reshape(*meta["x_shape"], d)


def kernel(x, packed, nf4_lut, c):
    nc, in_maps, meta = _prepare(x, packed, nf4_lut, c)
    res = bass_utils.run_bass_kernel_spmd(nc, in_maps, core_ids=list(range(N_CORES)))
    return _assemble(meta, [np.asarray(res.results[ci]["out"]) for ci in range(N_CORES)])


def _make_sharded(nc, in_maps):
    """Build a repeat-callable jitted 8-core executor for an already-compiled
    Bass program. Returns (call_fn, warm_outs_np)."""
    import jax
    import jax.numpy as jnp
    from jax.sharding import NamedSharding
    from concourse import bass2jax
    from concourse.bass2jax import Mesh, PartitionSpec, _bass_exec_p, shard_map

    bass2jax.install_neuronx_cc_hook()
    n_cores = len(in_maps)

    partition_name = nc.partition_id_tensor.name if nc.partition_id_tensor else None
    in_names, out_names, out_avals, zero_outs = [], [], [], []
    for alloc in nc.m.functions[0].allocations:
        if not isinstance(alloc, mybir.MemoryLocationSet):
            continue
        name = alloc.memorylocations[0].name
        if alloc.kind == "ExternalInput":
            if name != partition_name:
                in_names.append(name)
        elif alloc.kind == "ExternalOutput":
            out_names.append(name)
            shape = tuple(alloc.tensor_shape)
            dtype = mybir.dt.np(alloc.dtype)
            out_avals.append(jax.core.ShapedArray(shape, dtype))
            zero_outs.append(np.zeros(shape, dtype))
    n_params = len(in_names)
    n_outs = len(out_avals)
    all_in_names = list(in_names) + list(out_names)
    if partition_name is not None:
        all_in_names.append(partition_name)
    donate = tuple(range(n_params, n_params + n_outs))

    def _body(*args):
        operands = list(args)
        if partition_name is not None:
            operands.append(bass2jax.partition_id_tensor())
        outs = _bass_exec_p.bind(
            *operands,
            out_avals=tuple(out_avals),
            in_names=tuple(all_in_names),
            out_names=tuple(out_names),
            lowering_input_output_aliases=(),
            sim_require_finite=True,
            sim_require_nnan=True,
            nc=nc,
        )
        return tuple(outs)

    devices = jax.devices()[:n_cores]
    mesh = Mesh(np.asarray(devices), ("core",))
    in_specs = (PartitionSpec("core"),) * (n_params + n_outs)
    out_specs = (PartitionSpec("core"),) * n_outs
    sharded = jax.jit(
        shard_map(
            _body, mesh=mesh, in_specs=in_specs, out_specs=out_specs, check_rep=False
        ),
        donate_argnums=donate,
        keep_unused=True,
    )

    shard_across = NamedSharding(mesh, PartitionSpec("core"))
    concat_in = [
        np.concatenate([np.asarray(in_maps[ci][name]) for ci in range(n_cores)], axis=0)
        for name in in_names
    ]
    dev_in = [jax.device_put(a, shard_across) for a in concat_in]

    mkz = jax.jit(
        lambda: tuple(
            jnp.zeros((n_cores * z.shape[0], *z.shape[1:]), z.dtype) for z in zero_outs
        ),
        out_shardings=tuple(shard_across for _ in zero_outs),
    )

    def call():
        z = mkz()
        jax.block_until_ready(z)
        import time as _t

        t0 = _t.perf_counter()
        outs = sharded(*dev_in, *z)
        jax.block_until_ready(outs)
        return _t.perf_counter() - t0, outs

    _, warm = call()  # compile + warm
    warm_np = [np.asarray(w) for w in warm]
    return call, warm_np


def benchmark(x, packed, nf4_lut, c, reps=256, calls=16):
    """HW time via in-NEFF repetition: per-rep ns = (t(R) - t(1)) / (R - 1).
    The per-call dispatch overhead (~tens of ms under axon) is noisy, so use
    the min over `calls` executions of each program — the floor of a
    fixed-overhead + work distribution — rather than the median."""
    nc1, in_maps1, meta = _prepare(x, packed, nf4_lut, c, reps=1)
    call1, warm1 = _make_sharded(nc1, in_maps1)

    ncR, in_mapsR, _ = _prepare(x, packed, nf4_lut, c, reps=reps)
    callR, _ = _make_sharded(ncR, in_mapsR)

    import statistics

    s1, sR = [], []
    for _ in range(calls):
        s1.append(call1()[0])
        sR.append(callR()[0])
    # paired differences: t(R) and t(1) calls are adjacent in time, so
    # machine-load drift cancels in each pair before taking the median
    diffs = [b - a for a, b in zip(s1, sR)]
    ns = statistics.median(diffs) / (reps - 1) * 1e9
    min_ns = (min(sR) - min(s1)) / (reps - 1) * 1e9
    print(
        f"benchmark: med t(1)={statistics.median(s1) * 1e3:.3f}ms "
        f"med t({reps})={statistics.median(sR) * 1e3:.3f}ms "
        f"med paired diff={statistics.median(diffs) * 1e3:.3f}ms "
        f"-> {ns:.0f} ns/rep (min-based {min_ns:.0f})"
    )

    n_cores = len(in_maps1)
    per_core = warm1[0].reshape(n_cores, -1, meta["d"])
    result = _assemble(meta, [per_core[ci] for ci in range(n_cores)])
    return ns, result
